# revision 3
# baseline (speedup 1.0000x reference)
"""GCN message-passing kernel for 8 Trainium2 NeuronCores — v3.

dst-sharded (no collectives). Per core, edges are bucketed by destination
window (128 nodes) AND by src-table half (src < 32768 vs >= 32768, so row
indices fit dma_gather's int16), padded to 128-edge tiles. Gathering uses the
bulk InstDMAGatherAnt instruction (one call per (8-window group, half), ~50
tiles / 6400 row descriptors each) — this amortizes the ~1us/instr SWDGE cost
that bound v1. Data path is bf16 (half the gather bytes, 4x faster PE
matmuls); PSUM accumulation stays f32. The segment-sum is a weighted one-hot
matmul per tile; mean + isolated-node passthrough are folded into per-edge
weights. Output is written [D, NPC] bf16 (skips the PE transpose); the host
transposes/upcasts.
"""
import os
import sys
sys.path.insert(0, "/opt/trn_rl_repo")
os.environ.setdefault("NEURON_RT_RESET_CORES", "1")

import numpy as np
import concourse.bass as bass
import concourse.bacc as bacc
import concourse.mybir as mybir
import concourse.tile as tile
from concourse.bass_utils import run_bass_kernel_spmd

P = 128
N_NODES = 50000
N_EDGES = 600000
D = 128
N_CORES = 8
WIN = 128                      # nodes per window (= PSUM tile free size)
WINS_PER_CORE = 49             # 49 * 128 = 6272 node slots per core
NPC = WINS_PER_CORE * WIN
SPLIT = 32768                  # feature-table half boundary (int16 idx limit)
GG = 8                         # windows per gather group / output DMA block
PH = 4                         # windows per phase-2 (W matmul / relu) batch

BF16 = mybir.dt.bfloat16
NP_BF16 = mybir.dt.np(BF16)


def _host_schedule(feature, W, b, src, dst):
    """Shard + sort + pad edges; build per-core input tensors.

    Returns (in_maps, T_w, NT) where T_w is a flat int array of length
    2*WINS_PER_CORE: [T_wA..., T_wB...] (tiles per window per table half).
    """
    deg = np.bincount(dst, minlength=N_NODES).astype(np.int64)
    recip = 1.0 / np.maximum(deg, 1).astype(np.float32)

    iso = np.where(deg == 0)[0].astype(np.int64)
    if iso.size:
        src = np.concatenate([src, iso])
        dst = np.concatenate([dst, iso])
    E = src.shape[0]

    core = dst // NPC
    dst_local = dst - core * NPC
    win = dst_local >> 7
    half = (src >= SPLIT).astype(np.int64)

    cnt = np.zeros((N_CORES, WINS_PER_CORE, 2), dtype=np.int64)
    np.add.at(cnt, (core, win, half), 1)
    maxc = cnt.max(axis=0)                      # [WINS, 2]
    T_half = -(-maxc // P)                      # tiles per (window, half)
    # every window needs >= 1 tile so ph1 is always written
    emptyw = T_half.sum(axis=1) == 0
    T_half[emptyw, 0] = 1
    T_wA, T_wB = T_half[:, 0], T_half[:, 1]

    # global tile ids: per gather group of GG windows, A tiles then B tiles
    awbase = np.zeros(WINS_PER_CORE, dtype=np.int64)
    bwbase = np.zeros(WINS_PER_CORE, dtype=np.int64)
    gbase = []                                   # (tile base, nA, nB) per group
    t = 0
    for g0 in range(0, WINS_PER_CORE, GG):
        g1 = min(g0 + GG, WINS_PER_CORE)
        base = t
        for w in range(g0, g1):
            awbase[w] = t
            t += T_wA[w]
        nA = t - base
        for w in range(g0, g1):
            bwbase[w] = t
            t += T_wB[w]
        gbase.append((base, int(nA), int(t - base - nA)))
    NT = int(t)

    # slot for each edge: (core, win, half) bucket, sequential position
    order = np.lexsort((half, win, core))
    s_src = src[order]
    s_core = core[order]
    s_half = half[order]
    s_win = win[order]
    s_dloc = (dst_local[order] & 127).astype(np.float32)
    s_w = recip[dst[order]]

    grp = (s_core * WINS_PER_CORE + s_win) * 2 + s_half
    grp_starts = np.concatenate(
        [[0], np.cumsum(np.bincount(grp, minlength=N_CORES * WINS_PER_CORE * 2))])
    pos = np.arange(E) - grp_starts[grp]
    tile_base = np.where(s_half == 0, awbase[s_win], bwbase[s_win])
    slot = tile_base * P + pos

    idx16 = np.zeros((N_CORES, NT * P), dtype=np.int16)
    dlocs = np.zeros((N_CORES, NT * P), dtype=np.float32)
    ws = np.zeros((N_CORES, NT * P), dtype=np.float32)
    idx16[s_core, slot] = (s_src - s_half * SPLIT).astype(np.int16)
    dlocs[s_core, slot] = s_dloc
    ws[s_core, slot] = s_w

    feat_bf = np.ascontiguousarray(feature, dtype=np.float32).astype(NP_BF16)
    W_bf = np.ascontiguousarray(W, dtype=np.float32).astype(NP_BF16)
    bf = np.ascontiguousarray(b, dtype=np.float32).reshape(P, 1)
    in_maps = []
    for c in range(N_CORES):
        # idxs wrapped by 16: value of slot s at [s % 16, s // 16], replicated
        # across the 8 16-partition groups
        w16 = idx16[c].reshape(NT * 8, 16).T                 # [16, NT*8]
        idxbuf = np.tile(w16, (8, 1))                        # [128, NT*8]
        in_maps.append({
            "feat": feat_bf,
            "idxs": np.ascontiguousarray(idxbuf),
            "dstloc": np.ascontiguousarray(dlocs[c].reshape(NT, P).T),
            "wcol": np.ascontiguousarray(ws[c].reshape(NT, P).T),
            "Wmat": W_bf,
            "bias": bf,
        })
    T_w = np.concatenate([T_wA, T_wB]).astype(np.int64)
    return in_maps, T_w, NT


def _build(T_w, NT, reps=1, scratch=32768):
    T_w = np.asarray(T_w)
    T_wA, T_wB = T_w[:WINS_PER_CORE], T_w[WINS_PER_CORE:]

    # reconstruct group layout (same walk as _host_schedule)
    awbase = np.zeros(WINS_PER_CORE, dtype=np.int64)
    bwbase = np.zeros(WINS_PER_CORE, dtype=np.int64)
    groups = []                                  # (w0, w1, tile base, nA, nB)
    t = 0
    for g0 in range(0, WINS_PER_CORE, GG):
        g1 = min(g0 + GG, WINS_PER_CORE)
        base = t
        for w in range(g0, g1):
            awbase[w] = t
            t += int(T_wA[w])
        nA = t - base
        for w in range(g0, g1):
            bwbase[w] = t
            t += int(T_wB[w])
        groups.append((g0, g1, base, int(nA), int(t - base - nA)))
    assert t == NT
    GT_MAX = max(nA + nB for (_, _, _, nA, nB) in groups)

    nc = bacc.Bacc("TRN2", debug=False, num_devices=N_CORES,
                   dynamic_dma_scratch_size=scratch)
    feat = nc.dram_tensor("feat", [N_NODES, D], BF16, kind="ExternalInput")
    idxs = nc.dram_tensor("idxs", [P, NT * 8], mybir.dt.int16, kind="ExternalInput")
    dstloc = nc.dram_tensor("dstloc", [P, NT], mybir.dt.float32, kind="ExternalInput")
    wcol = nc.dram_tensor("wcol", [P, NT], mybir.dt.float32, kind="ExternalInput")
    Wmat = nc.dram_tensor("Wmat", [D, D], BF16, kind="ExternalInput")
    bias = nc.dram_tensor("bias", [P, 1], mybir.dt.float32, kind="ExternalInput")
    out = nc.dram_tensor("out", [D, NPC], BF16, kind="ExternalOutput")

    with tile.TileContext(nc) as tc:
        with (
            tc.tile_pool(name="const", bufs=1) as cpool,
            tc.tile_pool(name="msgs", bufs=2) as mpool,
            tc.tile_pool(name="oh", bufs=8) as ohpool,
            tc.tile_pool(name="hwin", bufs=2) as hpool,
            tc.tile_pool(name="outw", bufs=2) as opool,
            tc.tile_pool(name="ph1", bufs=2, space="PSUM") as p1pool,
            tc.tile_pool(name="ph2", bufs=2, space="PSUM") as p2pool,
        ):
            idx_t = cpool.tile([P, NT * 8], mybir.dt.int16)
            nc.sync.dma_start(idx_t[:], idxs[:])
            dst_t = cpool.tile([P, NT], mybir.dt.float32)
            nc.sync.dma_start(dst_t[:], dstloc[:])
            w_t = cpool.tile([P, NT], mybir.dt.float32)
            nc.sync.dma_start(w_t[:], wcol[:])
            Wt = cpool.tile([D, D], BF16)
            nc.sync.dma_start(Wt[:], Wmat[:])
            b_t = cpool.tile([P, 1], mybir.dt.float32)
            nc.sync.dma_start(b_t[:], bias[:])

            iota_i = cpool.tile([P, WIN], mybir.dt.int32)
            nc.gpsimd.iota(iota_i[:], pattern=[[1, WIN]], base=0, channel_multiplier=0)
            iota_b = cpool.tile([P, WIN], BF16)
            nc.vector.tensor_copy(iota_b[:], iota_i[:])

            for rep in range(reps):
                for (g0, g1, gtb, nA, nB) in groups:
                    m = mpool.tile([P, GT_MAX * D], BF16)
                    m3 = m[:].rearrange("p (t d) -> p t d", d=D)
                    if nA:
                        nc.gpsimd.dma_gather(
                            out_ap=m3[:, 0:nA, :], in_ap=feat[0:SPLIT, :],
                            idxs_ap=idx_t[:, gtb * 8:(gtb + nA) * 8],
                            num_idxs=nA * P, num_idxs_reg=nA * P,
                            elem_size=D, single_packet=False)
                    if nB:
                        nc.gpsimd.dma_gather(
                            out_ap=m3[:, nA:nA + nB, :], in_ap=feat[SPLIT:N_NODES, :],
                            idxs_ap=idx_t[:, (gtb + nA) * 8:(gtb + nA + nB) * 8],
                            num_idxs=nB * P, num_idxs_reg=nB * P,
                            elem_size=D, single_packet=False)

                    ow = opool.tile([P, GG * WIN], BF16)
                    for ph0 in range(g0, g1, PH):
                        ph1w = min(ph0 + PH, g1)
                        hT = hpool.tile([D, PH * WIN], BF16)
                        for w in range(ph0, ph1w):
                            gts = (list(range(int(awbase[w]), int(awbase[w] + T_wA[w])))
                                   + list(range(int(bwbase[w]), int(bwbase[w] + T_wB[w]))))
                            ph1 = p1pool.tile([D, WIN], mybir.dt.float32, space="PSUM")
                            for k, gt in enumerate(gts):
                                oh = ohpool.tile([P, WIN], BF16)
                                nc.vector.tensor_scalar(
                                    out=oh[:], in0=iota_b[:],
                                    scalar1=dst_t[:, gt:gt + 1],
                                    scalar2=w_t[:, gt:gt + 1],
                                    op0=mybir.AluOpType.is_equal,
                                    op1=mybir.AluOpType.mult,
                                )
                                j = gt - gtb
                                nc.tensor.matmul(
                                    out=ph1[:], lhsT=m[:, j * D:(j + 1) * D],
                                    rhs=oh[:], start=(k == 0),
                                    stop=(k == len(gts) - 1))
                            nc.vector.tensor_copy(
                                hT[:, (w - ph0) * WIN:(w - ph0 + 1) * WIN], ph1[:])
                        gw = ph1w - ph0
                        ph2 = p2pool.tile([D, PH * WIN], mybir.dt.float32, space="PSUM")
                        nc.tensor.matmul(out=ph2[:, :gw * WIN], lhsT=Wt[:],
                                         rhs=hT[:, :gw * WIN], start=True, stop=True)
                        nc.scalar.activation(
                            ow[:, (ph0 - g0) * WIN:(ph0 - g0 + gw) * WIN],
                            ph2[:, :gw * WIN],
                            mybir.ActivationFunctionType.Relu,
                            bias=b_t[:, 0:1], scale=1.0)
                    nc.sync.dma_start(out[:, g0 * WIN:g1 * WIN],
                                      ow[:, :(g1 - g0) * WIN])
    nc.compile()
    return nc


_CACHE = {}


def kernel(feature, W, b, src, dst):
    feature = np.asarray(feature, dtype=np.float32)
    W = np.asarray(W, dtype=np.float32)
    b = np.asarray(b, dtype=np.float32)
    src = np.asarray(src, dtype=np.int64)
    dst = np.asarray(dst, dtype=np.int64)

    in_maps, T_w, NT = _host_schedule(feature, W, b, src, dst)
    key = (NT, tuple(T_w.tolist()))
    if key not in _CACHE:
        _CACHE[key] = _build(T_w, NT)
    nc = _CACHE[key]
    res = run_bass_kernel_spmd(nc, in_maps, core_ids=list(range(N_CORES)))
    out = np.empty((N_NODES, D), dtype=np.float32)
    for c in range(N_CORES):
        lo = c * NPC
        hi = min(lo + NPC, N_NODES)
        oT = np.asarray(res.results[c]["out"]).astype(np.float32)  # [D, NPC]
        out[lo:hi] = oT.T[: hi - lo]
    return out


# revision 4
# speedup vs baseline: 2.2460x; 2.2460x over previous
"""GCN message-passing kernel for 8 Trainium2 NeuronCores — v3.

dst-sharded (no collectives). Per core, edges are bucketed by destination
window (128 nodes) AND by src-table half (src < 32768 vs >= 32768, so row
indices fit dma_gather's int16), padded to 128-edge tiles. Gathering uses the
bulk InstDMAGatherAnt instruction (one call per (8-window group, half), ~50
tiles / 6400 row descriptors each) — this amortizes the ~1us/instr SWDGE cost
that bound v1. Data path is bf16 (half the gather bytes, 4x faster PE
matmuls); PSUM accumulation stays f32. The segment-sum is a weighted one-hot
matmul per tile; mean + isolated-node passthrough are folded into per-edge
weights. Output is written [D, NPC] bf16 (skips the PE transpose); the host
transposes/upcasts.
"""
import os
import sys
sys.path.insert(0, "/opt/trn_rl_repo")
os.environ.setdefault("NEURON_RT_RESET_CORES", "1")

import numpy as np
import concourse.bass as bass
import concourse.bacc as bacc
import concourse.mybir as mybir
import concourse.tile as tile
from concourse.bass_utils import run_bass_kernel_spmd

P = 128
N_NODES = 50000
N_EDGES = 600000
D = 128
N_CORES = 8
WIN = 128                      # nodes per window (= PSUM tile free size)
WINS_PER_CORE = 49             # 49 * 128 = 6272 node slots per core
NPC = WINS_PER_CORE * WIN
SPLIT = 32768                  # feature-table half boundary (int16 idx limit)
GG = 8                         # windows per gather group / output DMA block
PH = 4                         # windows per phase-2 (W matmul / relu) batch

BF16 = mybir.dt.bfloat16
NP_BF16 = mybir.dt.np(BF16)


def _host_schedule(feature, W, b, src, dst):
    """Shard + sort + pad edges; build per-core input tensors.

    Returns (in_maps, T_w, NT) where T_w is a flat int array of length
    2*WINS_PER_CORE: [T_wA..., T_wB...] (tiles per window per table half).
    """
    deg = np.bincount(dst, minlength=N_NODES).astype(np.int64)
    recip = 1.0 / np.maximum(deg, 1).astype(np.float32)

    iso = np.where(deg == 0)[0].astype(np.int64)
    if iso.size:
        src = np.concatenate([src, iso])
        dst = np.concatenate([dst, iso])
    E = src.shape[0]

    core = dst // NPC
    dst_local = dst - core * NPC
    win = dst_local >> 7
    half = (src >= SPLIT).astype(np.int64)

    cnt = np.zeros((N_CORES, WINS_PER_CORE, 2), dtype=np.int64)
    np.add.at(cnt, (core, win, half), 1)
    maxc = cnt.max(axis=0)                      # [WINS, 2]
    T_half = -(-maxc // P)                      # tiles per (window, half)
    # every window needs >= 1 tile so ph1 is always written
    emptyw = T_half.sum(axis=1) == 0
    T_half[emptyw, 0] = 1
    T_wA, T_wB = T_half[:, 0], T_half[:, 1]

    # global tile ids: per gather group of GG windows, A tiles then B tiles
    awbase = np.zeros(WINS_PER_CORE, dtype=np.int64)
    bwbase = np.zeros(WINS_PER_CORE, dtype=np.int64)
    gbase = []                                   # (tile base, nA, nB) per group
    t = 0
    for g0 in range(0, WINS_PER_CORE, GG):
        g1 = min(g0 + GG, WINS_PER_CORE)
        base = t
        for w in range(g0, g1):
            awbase[w] = t
            t += T_wA[w]
        nA = t - base
        for w in range(g0, g1):
            bwbase[w] = t
            t += T_wB[w]
        gbase.append((base, int(nA), int(t - base - nA)))
    NT = int(t)

    # slot for each edge: (core, win, half) bucket, sequential position
    order = np.lexsort((half, win, core))
    s_src = src[order]
    s_core = core[order]
    s_half = half[order]
    s_win = win[order]
    s_dloc = (dst_local[order] & 127).astype(np.float32)
    s_w = recip[dst[order]]

    grp = (s_core * WINS_PER_CORE + s_win) * 2 + s_half
    grp_starts = np.concatenate(
        [[0], np.cumsum(np.bincount(grp, minlength=N_CORES * WINS_PER_CORE * 2))])
    pos = np.arange(E) - grp_starts[grp]
    tile_base = np.where(s_half == 0, awbase[s_win], bwbase[s_win])
    slot = tile_base * P + pos

    idx16 = np.zeros((N_CORES, NT * P), dtype=np.int16)
    dlocs = np.zeros((N_CORES, NT * P), dtype=np.float32)
    ws = np.zeros((N_CORES, NT * P), dtype=np.float32)
    idx16[s_core, slot] = (s_src - s_half * SPLIT).astype(np.int16)
    dlocs[s_core, slot] = s_dloc
    ws[s_core, slot] = s_w

    feat_bf = np.ascontiguousarray(feature, dtype=np.float32).astype(NP_BF16)
    W_bf = np.ascontiguousarray(W, dtype=np.float32).astype(NP_BF16)
    bf = np.ascontiguousarray(b, dtype=np.float32).reshape(P, 1)
    in_maps = []
    for c in range(N_CORES):
        # idxs wrapped by 16: value of slot s at [s % 16, s // 16], replicated
        # across the 8 16-partition groups
        w16 = idx16[c].reshape(NT * 8, 16).T                 # [16, NT*8]
        idxbuf = np.tile(w16, (8, 1))                        # [128, NT*8]
        in_maps.append({
            "feat": feat_bf,
            "idxs": np.ascontiguousarray(idxbuf),
            "dstloc": np.ascontiguousarray(dlocs[c].reshape(NT, P).T),
            "wcol": np.ascontiguousarray(ws[c].reshape(NT, P).T),
            "Wmat": W_bf,
            "bias": bf,
        })
    T_w = np.concatenate([T_wA, T_wB]).astype(np.int64)
    return in_maps, T_w, NT


def _build(T_w, NT, reps=1, scratch=32768):
    T_w = np.asarray(T_w)
    T_wA, T_wB = T_w[:WINS_PER_CORE], T_w[WINS_PER_CORE:]

    # reconstruct group layout (same walk as _host_schedule)
    awbase = np.zeros(WINS_PER_CORE, dtype=np.int64)
    bwbase = np.zeros(WINS_PER_CORE, dtype=np.int64)
    groups = []                                  # (w0, w1, tile base, nA, nB)
    t = 0
    for g0 in range(0, WINS_PER_CORE, GG):
        g1 = min(g0 + GG, WINS_PER_CORE)
        base = t
        for w in range(g0, g1):
            awbase[w] = t
            t += int(T_wA[w])
        nA = t - base
        for w in range(g0, g1):
            bwbase[w] = t
            t += int(T_wB[w])
        groups.append((g0, g1, base, int(nA), int(t - base - nA)))
    assert t == NT
    GT_MAX = max(nA + nB for (_, _, _, nA, nB) in groups)

    nc = bacc.Bacc("TRN2", debug=False, num_devices=N_CORES,
                   dynamic_dma_scratch_size=scratch, num_swdge_queues=4)
    feat = nc.dram_tensor("feat", [N_NODES, D], BF16, kind="ExternalInput")
    idxs = nc.dram_tensor("idxs", [P, NT * 8], mybir.dt.int16, kind="ExternalInput")
    dstloc = nc.dram_tensor("dstloc", [P, NT], mybir.dt.float32, kind="ExternalInput")
    wcol = nc.dram_tensor("wcol", [P, NT], mybir.dt.float32, kind="ExternalInput")
    Wmat = nc.dram_tensor("Wmat", [D, D], BF16, kind="ExternalInput")
    bias = nc.dram_tensor("bias", [P, 1], mybir.dt.float32, kind="ExternalInput")
    out = nc.dram_tensor("out", [D, NPC], BF16, kind="ExternalOutput")

    with tile.TileContext(nc) as tc:
        with (
            tc.tile_pool(name="const", bufs=1) as cpool,
            tc.tile_pool(name="msgs", bufs=2) as mpool,
            tc.tile_pool(name="oh", bufs=8) as ohpool,
            tc.tile_pool(name="hwin", bufs=2) as hpool,
            tc.tile_pool(name="outw", bufs=2) as opool,
            tc.tile_pool(name="ph1", bufs=2, space="PSUM") as p1pool,
            tc.tile_pool(name="ph2", bufs=2, space="PSUM") as p2pool,
        ):
            idx_t = cpool.tile([P, NT * 8], mybir.dt.int16)
            nc.sync.dma_start(idx_t[:], idxs[:])
            dst_t = cpool.tile([P, NT], mybir.dt.float32)
            nc.sync.dma_start(dst_t[:], dstloc[:])
            w_t = cpool.tile([P, NT], mybir.dt.float32)
            nc.sync.dma_start(w_t[:], wcol[:])
            Wt = cpool.tile([D, D], BF16)
            nc.sync.dma_start(Wt[:], Wmat[:])
            b_t = cpool.tile([P, 1], mybir.dt.float32)
            nc.sync.dma_start(b_t[:], bias[:])

            iota_i = cpool.tile([P, WIN], mybir.dt.int32)
            nc.gpsimd.iota(iota_i[:], pattern=[[1, WIN]], base=0, channel_multiplier=0)
            iota_b = cpool.tile([P, WIN], BF16)
            nc.vector.tensor_copy(iota_b[:], iota_i[:])

            qn = 0
            for rep in range(reps):
                for (g0, g1, gtb, nA, nB) in groups:
                    m = mpool.tile([P, GT_MAX * D], BF16)
                    m3 = m[:].rearrange("p (t d) -> p t d", d=D)
                    # one gather per (window, half), rotated over the 4 SWDGE
                    # queues so Q7 descriptor generation runs 4-wide
                    for w in range(g0, g1):
                        for (base, hi, t0, tw) in (
                            (0, SPLIT, int(awbase[w]), int(T_wA[w])),
                            (SPLIT, N_NODES, int(bwbase[w]), int(T_wB[w])),
                        ):
                            if not tw:
                                continue
                            j0 = t0 - gtb
                            nc.gpsimd.dma_gather(
                                out_ap=m3[:, j0:j0 + tw, :],
                                in_ap=feat[base:hi, :],
                                idxs_ap=idx_t[:, t0 * 8:(t0 + tw) * 8],
                                num_idxs=tw * P, num_idxs_reg=tw * P,
                                elem_size=D, single_packet=False,
                                queue_num=qn % 4)
                            qn += 1

                    ow = opool.tile([P, GG * WIN], BF16)
                    for ph0 in range(g0, g1, PH):
                        ph1w = min(ph0 + PH, g1)
                        hT = hpool.tile([D, PH * WIN], BF16)
                        for w in range(ph0, ph1w):
                            gts = (list(range(int(awbase[w]), int(awbase[w] + T_wA[w])))
                                   + list(range(int(bwbase[w]), int(bwbase[w] + T_wB[w]))))
                            ph1 = p1pool.tile([D, WIN], mybir.dt.float32, space="PSUM")
                            for k, gt in enumerate(gts):
                                oh = ohpool.tile([P, WIN], BF16)
                                nc.vector.tensor_scalar(
                                    out=oh[:], in0=iota_b[:],
                                    scalar1=dst_t[:, gt:gt + 1],
                                    scalar2=w_t[:, gt:gt + 1],
                                    op0=mybir.AluOpType.is_equal,
                                    op1=mybir.AluOpType.mult,
                                )
                                j = gt - gtb
                                nc.tensor.matmul(
                                    out=ph1[:], lhsT=m[:, j * D:(j + 1) * D],
                                    rhs=oh[:], start=(k == 0),
                                    stop=(k == len(gts) - 1))
                            nc.vector.tensor_copy(
                                hT[:, (w - ph0) * WIN:(w - ph0 + 1) * WIN], ph1[:])
                        gw = ph1w - ph0
                        ph2 = p2pool.tile([D, PH * WIN], mybir.dt.float32, space="PSUM")
                        nc.tensor.matmul(out=ph2[:, :gw * WIN], lhsT=Wt[:],
                                         rhs=hT[:, :gw * WIN], start=True, stop=True)
                        nc.scalar.activation(
                            ow[:, (ph0 - g0) * WIN:(ph0 - g0 + gw) * WIN],
                            ph2[:, :gw * WIN],
                            mybir.ActivationFunctionType.Relu,
                            bias=b_t[:, 0:1], scale=1.0)
                    nc.sync.dma_start(out[:, g0 * WIN:g1 * WIN],
                                      ow[:, :(g1 - g0) * WIN])
    nc.compile()
    return nc


_CACHE = {}


def kernel(feature, W, b, src, dst):
    feature = np.asarray(feature, dtype=np.float32)
    W = np.asarray(W, dtype=np.float32)
    b = np.asarray(b, dtype=np.float32)
    src = np.asarray(src, dtype=np.int64)
    dst = np.asarray(dst, dtype=np.int64)

    in_maps, T_w, NT = _host_schedule(feature, W, b, src, dst)
    key = (NT, tuple(T_w.tolist()))
    if key not in _CACHE:
        _CACHE[key] = _build(T_w, NT)
    nc = _CACHE[key]
    res = run_bass_kernel_spmd(nc, in_maps, core_ids=list(range(N_CORES)))
    out = np.empty((N_NODES, D), dtype=np.float32)
    for c in range(N_CORES):
        lo = c * NPC
        hi = min(lo + NPC, N_NODES)
        oT = np.asarray(res.results[c]["out"]).astype(np.float32)  # [D, NPC]
        out[lo:hi] = oT.T[: hi - lo]
    return out


# revision 5
# speedup vs baseline: 2.3382x; 1.0410x over previous
"""GCN message-passing kernel for 8 Trainium2 NeuronCores — v3.

dst-sharded (no collectives). Per core, edges are bucketed by destination
window (128 nodes) AND by src-table half (src < 32768 vs >= 32768, so row
indices fit dma_gather's int16), padded to 128-edge tiles. Gathering uses the
bulk InstDMAGatherAnt instruction (one call per (8-window group, half), ~50
tiles / 6400 row descriptors each) — this amortizes the ~1us/instr SWDGE cost
that bound v1. Data path is bf16 (half the gather bytes, 4x faster PE
matmuls); PSUM accumulation stays f32. The segment-sum is a weighted one-hot
matmul per tile; mean + isolated-node passthrough are folded into per-edge
weights. Output is written [D, NPC] bf16 (skips the PE transpose); the host
transposes/upcasts.
"""
import os
import sys
sys.path.insert(0, "/opt/trn_rl_repo")
os.environ.setdefault("NEURON_RT_RESET_CORES", "1")

import numpy as np
import concourse.bass as bass
import concourse.bacc as bacc
import concourse.mybir as mybir
import concourse.tile as tile
from concourse.bass_utils import run_bass_kernel_spmd

P = 128
N_NODES = 50000
N_EDGES = 600000
D = 128
N_CORES = 8
WIN = 128                      # nodes per window (= PSUM tile free size)
WINS_PER_CORE = 49             # 49 * 128 = 6272 node slots per core
NPC = WINS_PER_CORE * WIN
SPLIT = 32768                  # feature-table half boundary (int16 idx limit)
GG = 8                         # windows per gather group / output DMA block
PH = 4                         # windows per phase-2 (W matmul / relu) batch

BF16 = mybir.dt.bfloat16
NP_BF16 = mybir.dt.np(BF16)


def _host_schedule(feature, W, b, src, dst):
    """Shard + sort + pad edges; build per-core input tensors.

    Returns (in_maps, T_w, NT) where T_w is a flat int array of length
    2*WINS_PER_CORE: [T_wA..., T_wB...] (tiles per window per table half).
    """
    deg = np.bincount(dst, minlength=N_NODES).astype(np.int64)
    recip = 1.0 / np.maximum(deg, 1).astype(np.float32)

    iso = np.where(deg == 0)[0].astype(np.int64)
    if iso.size:
        src = np.concatenate([src, iso])
        dst = np.concatenate([dst, iso])
    E = src.shape[0]

    core = dst // NPC
    dst_local = dst - core * NPC
    win = dst_local >> 7
    half = (src >= SPLIT).astype(np.int64)

    cnt = np.zeros((N_CORES, WINS_PER_CORE, 2), dtype=np.int64)
    np.add.at(cnt, (core, win, half), 1)
    maxc = cnt.max(axis=0)                      # [WINS, 2]
    T_half = -(-maxc // P)                      # tiles per (window, half)
    # every window needs >= 1 tile so ph1 is always written
    emptyw = T_half.sum(axis=1) == 0
    T_half[emptyw, 0] = 1
    T_wA, T_wB = T_half[:, 0], T_half[:, 1]

    # global tile ids: per gather group of GG windows, A tiles then B tiles
    awbase = np.zeros(WINS_PER_CORE, dtype=np.int64)
    bwbase = np.zeros(WINS_PER_CORE, dtype=np.int64)
    gbase = []                                   # (tile base, nA, nB) per group
    t = 0
    for g0 in range(0, WINS_PER_CORE, GG):
        g1 = min(g0 + GG, WINS_PER_CORE)
        base = t
        for w in range(g0, g1):
            awbase[w] = t
            t += T_wA[w]
        nA = t - base
        for w in range(g0, g1):
            bwbase[w] = t
            t += T_wB[w]
        gbase.append((base, int(nA), int(t - base - nA)))
    NT = int(t)

    # slot for each edge: (core, win, half) bucket, sequential position
    order = np.lexsort((half, win, core))
    s_src = src[order]
    s_core = core[order]
    s_half = half[order]
    s_win = win[order]
    s_dloc = (dst_local[order] & 127).astype(np.float32)
    s_w = recip[dst[order]]

    grp = (s_core * WINS_PER_CORE + s_win) * 2 + s_half
    grp_starts = np.concatenate(
        [[0], np.cumsum(np.bincount(grp, minlength=N_CORES * WINS_PER_CORE * 2))])
    pos = np.arange(E) - grp_starts[grp]
    tile_base = np.where(s_half == 0, awbase[s_win], bwbase[s_win])
    slot = tile_base * P + pos

    # pad slots must gather SOMETHING; identical indices back-to-back are
    # pathologically slow on the SDMA path, so give pads distinct row indices
    # (valid for either table half: < N_NODES - SPLIT)
    pad_pattern = (np.arange(NT * P, dtype=np.int64) % (N_NODES - SPLIT)).astype(np.int16)
    idx16 = np.tile(pad_pattern, (N_CORES, 1))
    dlocs = np.zeros((N_CORES, NT * P), dtype=np.float32)
    ws = np.zeros((N_CORES, NT * P), dtype=np.float32)
    idx16[s_core, slot] = (s_src - s_half * SPLIT).astype(np.int16)
    dlocs[s_core, slot] = s_dloc
    ws[s_core, slot] = s_w

    feat_bf = np.ascontiguousarray(feature, dtype=np.float32).astype(NP_BF16)
    W_bf = np.ascontiguousarray(W, dtype=np.float32).astype(NP_BF16)
    bf = np.ascontiguousarray(b, dtype=np.float32).reshape(P, 1)
    in_maps = []
    for c in range(N_CORES):
        # idxs wrapped by 16: value of slot s at [s % 16, s // 16], replicated
        # across the 8 16-partition groups
        w16 = idx16[c].reshape(NT * 8, 16).T                 # [16, NT*8]
        idxbuf = np.tile(w16, (8, 1))                        # [128, NT*8]
        in_maps.append({
            "feat": feat_bf,
            "idxs": np.ascontiguousarray(idxbuf),
            "dstloc": np.ascontiguousarray(dlocs[c].reshape(NT, P).T),
            "wcol": np.ascontiguousarray(ws[c].reshape(NT, P).T),
            "Wmat": W_bf,
            "bias": bf,
        })
    T_w = np.concatenate([T_wA, T_wB]).astype(np.int64)
    return in_maps, T_w, NT


def _build(T_w, NT, reps=1, scratch=32768, mode="full"):
    T_w = np.asarray(T_w)
    T_wA, T_wB = T_w[:WINS_PER_CORE], T_w[WINS_PER_CORE:]

    # reconstruct group layout (same walk as _host_schedule)
    awbase = np.zeros(WINS_PER_CORE, dtype=np.int64)
    bwbase = np.zeros(WINS_PER_CORE, dtype=np.int64)
    groups = []                                  # (w0, w1, tile base, nA, nB)
    t = 0
    for g0 in range(0, WINS_PER_CORE, GG):
        g1 = min(g0 + GG, WINS_PER_CORE)
        base = t
        for w in range(g0, g1):
            awbase[w] = t
            t += int(T_wA[w])
        nA = t - base
        for w in range(g0, g1):
            bwbase[w] = t
            t += int(T_wB[w])
        groups.append((g0, g1, base, int(nA), int(t - base - nA)))
    assert t == NT
    GT_MAX = max(nA + nB for (_, _, _, nA, nB) in groups)

    nc = bacc.Bacc("TRN2", debug=False, num_devices=N_CORES,
                   dynamic_dma_scratch_size=scratch, num_swdge_queues=4)
    feat = nc.dram_tensor("feat", [N_NODES, D], BF16, kind="ExternalInput")
    idxs = nc.dram_tensor("idxs", [P, NT * 8], mybir.dt.int16, kind="ExternalInput")
    dstloc = nc.dram_tensor("dstloc", [P, NT], mybir.dt.float32, kind="ExternalInput")
    wcol = nc.dram_tensor("wcol", [P, NT], mybir.dt.float32, kind="ExternalInput")
    Wmat = nc.dram_tensor("Wmat", [D, D], BF16, kind="ExternalInput")
    bias = nc.dram_tensor("bias", [P, 1], mybir.dt.float32, kind="ExternalInput")
    out = nc.dram_tensor("out", [D, NPC], BF16, kind="ExternalOutput")

    with tile.TileContext(nc) as tc:
        with (
            tc.tile_pool(name="const", bufs=1) as cpool,
            tc.tile_pool(name="msgs", bufs=2) as mpool,
            tc.tile_pool(name="oh", bufs=8) as ohpool,
            tc.tile_pool(name="hwin", bufs=2) as hpool,
            tc.tile_pool(name="outw", bufs=2) as opool,
            tc.tile_pool(name="ph1", bufs=2, space="PSUM") as p1pool,
            tc.tile_pool(name="ph2", bufs=2, space="PSUM") as p2pool,
        ):
            idx_t = cpool.tile([P, NT * 8], mybir.dt.int16)
            nc.sync.dma_start(idx_t[:], idxs[:])
            dst_t = cpool.tile([P, NT], mybir.dt.float32)
            nc.sync.dma_start(dst_t[:], dstloc[:])
            w_t = cpool.tile([P, NT], mybir.dt.float32)
            nc.sync.dma_start(w_t[:], wcol[:])
            Wt = cpool.tile([D, D], BF16)
            nc.sync.dma_start(Wt[:], Wmat[:])
            b_t = cpool.tile([P, 1], mybir.dt.float32)
            nc.sync.dma_start(b_t[:], bias[:])

            iota_i = cpool.tile([P, WIN], mybir.dt.int32)
            nc.gpsimd.iota(iota_i[:], pattern=[[1, WIN]], base=0, channel_multiplier=0)
            iota_b = cpool.tile([P, WIN], BF16)
            nc.vector.tensor_copy(iota_b[:], iota_i[:])

            qn = 0
            for rep in range(reps):
                for (g0, g1, gtb, nA, nB) in groups:
                    m = mpool.tile([P, GT_MAX * D], BF16)
                    m3 = m[:].rearrange("p (t d) -> p t d", d=D)
                    # one gather per (window, half), rotated over the 4 SWDGE
                    # queues so Q7 descriptor generation runs 4-wide
                    for w in range(g0, g1):
                        for (base, hi, t0, tw) in (
                            (0, SPLIT, int(awbase[w]), int(T_wA[w])),
                            (SPLIT, N_NODES, int(bwbase[w]), int(T_wB[w])),
                        ):
                            if not tw or mode == "compute":
                                continue
                            j0 = t0 - gtb
                            nc.gpsimd.dma_gather(
                                out_ap=m3[:, j0:j0 + tw, :],
                                in_ap=feat[base:hi, :],
                                idxs_ap=idx_t[:, t0 * 8:(t0 + tw) * 8],
                                num_idxs=tw * P, num_idxs_reg=tw * P,
                                elem_size=D, single_packet=False,
                                queue_num=qn % 4)
                            qn += 1

                    if mode == "gather":
                        if g1 == WINS_PER_CORE:
                            nc.sync.dma_start(out[:, 0:GG * WIN],
                                              m[:, : GG * WIN])
                        continue
                    ow = opool.tile([P, GG * WIN], BF16)
                    for ph0 in range(g0, g1, PH):
                        ph1w = min(ph0 + PH, g1)
                        hT = hpool.tile([D, PH * WIN], BF16)
                        for w in range(ph0, ph1w):
                            gts = (list(range(int(awbase[w]), int(awbase[w] + T_wA[w])))
                                   + list(range(int(bwbase[w]), int(bwbase[w] + T_wB[w]))))
                            ph1 = p1pool.tile([D, WIN], mybir.dt.float32, space="PSUM")
                            for k, gt in enumerate(gts):
                                oh = ohpool.tile([P, WIN], BF16)
                                nc.vector.tensor_scalar(
                                    out=oh[:], in0=iota_b[:],
                                    scalar1=dst_t[:, gt:gt + 1],
                                    scalar2=w_t[:, gt:gt + 1],
                                    op0=mybir.AluOpType.is_equal,
                                    op1=mybir.AluOpType.mult,
                                )
                                j = gt - gtb
                                nc.tensor.matmul(
                                    out=ph1[:], lhsT=m[:, j * D:(j + 1) * D],
                                    rhs=oh[:], start=(k == 0),
                                    stop=(k == len(gts) - 1))
                            nc.vector.tensor_copy(
                                hT[:, (w - ph0) * WIN:(w - ph0 + 1) * WIN], ph1[:])
                        gw = ph1w - ph0
                        ph2 = p2pool.tile([D, PH * WIN], mybir.dt.float32, space="PSUM")
                        nc.tensor.matmul(out=ph2[:, :gw * WIN], lhsT=Wt[:],
                                         rhs=hT[:, :gw * WIN], start=True, stop=True)
                        nc.scalar.activation(
                            ow[:, (ph0 - g0) * WIN:(ph0 - g0 + gw) * WIN],
                            ph2[:, :gw * WIN],
                            mybir.ActivationFunctionType.Relu,
                            bias=b_t[:, 0:1], scale=1.0)
                    nc.sync.dma_start(out[:, g0 * WIN:g1 * WIN],
                                      ow[:, :(g1 - g0) * WIN])
    nc.compile()
    return nc


_CACHE = {}


def kernel(feature, W, b, src, dst):
    feature = np.asarray(feature, dtype=np.float32)
    W = np.asarray(W, dtype=np.float32)
    b = np.asarray(b, dtype=np.float32)
    src = np.asarray(src, dtype=np.int64)
    dst = np.asarray(dst, dtype=np.int64)

    in_maps, T_w, NT = _host_schedule(feature, W, b, src, dst)
    key = (NT, tuple(T_w.tolist()))
    if key not in _CACHE:
        _CACHE[key] = _build(T_w, NT)
    nc = _CACHE[key]
    res = run_bass_kernel_spmd(nc, in_maps, core_ids=list(range(N_CORES)))
    out = np.empty((N_NODES, D), dtype=np.float32)
    for c in range(N_CORES):
        lo = c * NPC
        hi = min(lo + NPC, N_NODES)
        oT = np.asarray(res.results[c]["out"]).astype(np.float32)  # [D, NPC]
        out[lo:hi] = oT.T[: hi - lo]
    return out


# revision 6
# speedup vs baseline: 2.6520x; 1.1342x over previous
"""GCN message-passing kernel for 8 Trainium2 NeuronCores — v3.

dst-sharded (no collectives). Per core, edges are bucketed by destination
window (128 nodes) AND by src-table half (src < 32768 vs >= 32768, so row
indices fit dma_gather's int16), padded to 128-edge tiles. Gathering uses the
bulk InstDMAGatherAnt instruction (one call per (8-window group, half), ~50
tiles / 6400 row descriptors each) — this amortizes the ~1us/instr SWDGE cost
that bound v1. Data path is bf16 (half the gather bytes, 4x faster PE
matmuls); PSUM accumulation stays f32. The segment-sum is a weighted one-hot
matmul per tile; mean + isolated-node passthrough are folded into per-edge
weights. Output is written [D, NPC] bf16 (skips the PE transpose); the host
transposes/upcasts.
"""
import os
import sys
sys.path.insert(0, "/opt/trn_rl_repo")
os.environ.setdefault("NEURON_RT_RESET_CORES", "1")

import numpy as np
import concourse.bass as bass
import concourse.bacc as bacc
import concourse.mybir as mybir
import concourse.tile as tile
from concourse.bass_utils import run_bass_kernel_spmd

P = 128
N_NODES = 50000
N_EDGES = 600000
D = 128
N_CORES = 8
WIN = 128                      # nodes per window (= PSUM tile free size)
WINS_PER_CORE = 49             # 49 * 128 = 6272 node slots per core
NPC = WINS_PER_CORE * WIN
SPLIT = 32768                  # feature-table half boundary (int16 idx limit)
GG = 8                         # windows per gather group / output DMA block
PH = 4                         # windows per phase-2 (W matmul / relu) batch

BF16 = mybir.dt.bfloat16
NP_BF16 = mybir.dt.np(BF16)


def _host_schedule(feature, W, b, src, dst):
    """Shard + sort + pad edges; build per-core input tensors.

    Returns (in_maps, T_w, NT) where T_w is a flat int array of length
    2*WINS_PER_CORE: [T_wA..., T_wB...] (tiles per window per table half).
    """
    deg = np.bincount(dst, minlength=N_NODES).astype(np.int64)
    recip = 1.0 / np.maximum(deg, 1).astype(np.float32)

    iso = np.where(deg == 0)[0].astype(np.int64)
    if iso.size:
        src = np.concatenate([src, iso])
        dst = np.concatenate([dst, iso])
    E = src.shape[0]

    core = dst // NPC
    dst_local = dst - core * NPC
    win = dst_local >> 7
    half = (src >= SPLIT).astype(np.int64)

    cnt = np.zeros((N_CORES, WINS_PER_CORE, 2), dtype=np.int64)
    np.add.at(cnt, (core, win, half), 1)
    maxc = cnt.max(axis=0)                      # [WINS, 2]
    T_half = -(-maxc // P)                      # tiles per (window, half)
    # every window needs >= 1 tile so ph1 is always written
    emptyw = T_half.sum(axis=1) == 0
    T_half[emptyw, 0] = 1
    T_wA, T_wB = T_half[:, 0], T_half[:, 1]

    # global tile ids: per gather group of GG windows, A tiles then B tiles
    awbase = np.zeros(WINS_PER_CORE, dtype=np.int64)
    bwbase = np.zeros(WINS_PER_CORE, dtype=np.int64)
    gbase = []                                   # (tile base, nA, nB) per group
    t = 0
    for g0 in range(0, WINS_PER_CORE, GG):
        g1 = min(g0 + GG, WINS_PER_CORE)
        base = t
        for w in range(g0, g1):
            awbase[w] = t
            t += T_wA[w]
        nA = t - base
        for w in range(g0, g1):
            bwbase[w] = t
            t += T_wB[w]
        gbase.append((base, int(nA), int(t - base - nA)))
    NT = int(t)

    # slot for each edge: (core, win, half) bucket, sequential position
    order = np.lexsort((half, win, core))
    s_src = src[order]
    s_core = core[order]
    s_half = half[order]
    s_win = win[order]
    s_dloc = (dst_local[order] & 127).astype(np.float32)
    s_w = recip[dst[order]]

    grp = (s_core * WINS_PER_CORE + s_win) * 2 + s_half
    grp_starts = np.concatenate(
        [[0], np.cumsum(np.bincount(grp, minlength=N_CORES * WINS_PER_CORE * 2))])
    pos = np.arange(E) - grp_starts[grp]
    tile_base = np.where(s_half == 0, awbase[s_win], bwbase[s_win])
    slot = tile_base * P + pos

    # pad slots must gather SOMETHING; identical indices back-to-back are
    # pathologically slow on the SDMA path, so give pads distinct row indices
    # (valid for either table half: < N_NODES - SPLIT)
    pad_pattern = (np.arange(NT * P, dtype=np.int64) % (N_NODES - SPLIT)).astype(np.int16)
    idx16 = np.tile(pad_pattern, (N_CORES, 1))
    dlocs = np.zeros((N_CORES, NT * P), dtype=np.float32)
    ws = np.zeros((N_CORES, NT * P), dtype=np.float32)
    idx16[s_core, slot] = (s_src - s_half * SPLIT).astype(np.int16)
    dlocs[s_core, slot] = s_dloc
    ws[s_core, slot] = s_w

    feat_bf = np.ascontiguousarray(feature, dtype=np.float32).astype(NP_BF16)
    W_bf = np.ascontiguousarray(W, dtype=np.float32).astype(NP_BF16)
    bf = np.ascontiguousarray(b, dtype=np.float32).reshape(P, 1)
    in_maps = []
    for c in range(N_CORES):
        # idxs wrapped by 16: value of slot s at [s % 16, s // 16], replicated
        # across the 8 16-partition groups
        w16 = idx16[c].reshape(NT * 8, 16).T                 # [16, NT*8]
        idxbuf = np.tile(w16, (8, 1))                        # [128, NT*8]
        in_maps.append({
            "feat": feat_bf,
            "idxs": np.ascontiguousarray(idxbuf),
            "dstloc": np.ascontiguousarray(dlocs[c].reshape(NT, P).T),
            "wcol": np.ascontiguousarray(ws[c].reshape(NT, P).T),
            "Wmat": W_bf,
            "bias": bf,
        })
    T_w = np.concatenate([T_wA, T_wB]).astype(np.int64)
    return in_maps, T_w, NT


def _build(T_w, NT, reps=1, scratch=32768, mode="full"):
    T_w = np.asarray(T_w)
    T_wA, T_wB = T_w[:WINS_PER_CORE], T_w[WINS_PER_CORE:]

    # reconstruct group layout (same walk as _host_schedule)
    awbase = np.zeros(WINS_PER_CORE, dtype=np.int64)
    bwbase = np.zeros(WINS_PER_CORE, dtype=np.int64)
    groups = []                                  # (w0, w1, tile base, nA, nB)
    t = 0
    for g0 in range(0, WINS_PER_CORE, GG):
        g1 = min(g0 + GG, WINS_PER_CORE)
        base = t
        for w in range(g0, g1):
            awbase[w] = t
            t += int(T_wA[w])
        nA = t - base
        for w in range(g0, g1):
            bwbase[w] = t
            t += int(T_wB[w])
        groups.append((g0, g1, base, int(nA), int(t - base - nA)))
    assert t == NT
    GT_MAX = max(nA + nB for (_, _, _, nA, nB) in groups)

    nc = bacc.Bacc("TRN2", debug=False, num_devices=N_CORES,
                   dynamic_dma_scratch_size=scratch, num_swdge_queues=4)
    feat = nc.dram_tensor("feat", [N_NODES, D], BF16, kind="ExternalInput")
    idxs = nc.dram_tensor("idxs", [P, NT * 8], mybir.dt.int16, kind="ExternalInput")
    dstloc = nc.dram_tensor("dstloc", [P, NT], mybir.dt.float32, kind="ExternalInput")
    wcol = nc.dram_tensor("wcol", [P, NT], mybir.dt.float32, kind="ExternalInput")
    Wmat = nc.dram_tensor("Wmat", [D, D], BF16, kind="ExternalInput")
    bias = nc.dram_tensor("bias", [P, 1], mybir.dt.float32, kind="ExternalInput")
    out = nc.dram_tensor("out", [D, NPC], BF16, kind="ExternalOutput")

    with tile.TileContext(nc) as tc:
        with (
            tc.tile_pool(name="const", bufs=1) as cpool,
            tc.tile_pool(name="msgs", bufs=3) as mpool,
            tc.tile_pool(name="oh", bufs=16) as ohpool,
            tc.tile_pool(name="hwin", bufs=2) as hpool,
            tc.tile_pool(name="outw", bufs=2) as opool,
            tc.tile_pool(name="ph1", bufs=4, space="PSUM") as p1pool,
            tc.tile_pool(name="ph2", bufs=2, space="PSUM") as p2pool,
        ):
            idx_t = cpool.tile([P, NT * 8], mybir.dt.int16)
            nc.sync.dma_start(idx_t[:], idxs[:])
            dst_t = cpool.tile([P, NT], mybir.dt.float32)
            nc.sync.dma_start(dst_t[:], dstloc[:])
            w_t = cpool.tile([P, NT], mybir.dt.float32)
            nc.sync.dma_start(w_t[:], wcol[:])
            Wt = cpool.tile([D, D], BF16)
            nc.sync.dma_start(Wt[:], Wmat[:])
            b_t = cpool.tile([P, 1], mybir.dt.float32)
            nc.sync.dma_start(b_t[:], bias[:])

            iota_i = cpool.tile([P, WIN], mybir.dt.int32)
            nc.gpsimd.iota(iota_i[:], pattern=[[1, WIN]], base=0, channel_multiplier=0)
            iota_b = cpool.tile([P, WIN], BF16)
            nc.vector.tensor_copy(iota_b[:], iota_i[:])

            if mode == "compute":
                mconst = cpool.tile([P, GT_MAX * D], BF16)
                nc.gpsimd.memset(mconst[:], 0.25)

            qn = 0
            for rep in range(reps):
                for (g0, g1, gtb, nA, nB) in groups:
                    if mode == "compute":
                        m = mconst
                    else:
                        m = mpool.tile([P, GT_MAX * D], BF16)
                    m3 = m[:].rearrange("p (t d) -> p t d", d=D)
                    # one gather per (window, half), rotated over the 4 SWDGE
                    # queues so Q7 descriptor generation runs 4-wide
                    for w in range(g0, g1):
                        for (base, hi, t0, tw) in (
                            (0, SPLIT, int(awbase[w]), int(T_wA[w])),
                            (SPLIT, N_NODES, int(bwbase[w]), int(T_wB[w])),
                        ):
                            if not tw or mode == "compute":
                                continue
                            j0 = t0 - gtb
                            nc.gpsimd.dma_gather(
                                out_ap=m3[:, j0:j0 + tw, :],
                                in_ap=feat[base:hi, :],
                                idxs_ap=idx_t[:, t0 * 8:(t0 + tw) * 8],
                                num_idxs=tw * P, num_idxs_reg=tw * P,
                                elem_size=D, single_packet=False,
                                queue_num=qn % 4)
                            qn += 1

                    if mode == "gather":
                        if g1 == WINS_PER_CORE:
                            nc.sync.dma_start(out[:, 0:GG * WIN],
                                              m[:, : GG * WIN])
                        continue
                    ow = opool.tile([P, GG * WIN], BF16)
                    for ph0 in range(g0, g1, PH):
                        ph1w = min(ph0 + PH, g1)
                        hT = hpool.tile([D, PH * WIN], BF16)
                        for w in range(ph0, ph1w):
                            gts = (list(range(int(awbase[w]), int(awbase[w] + T_wA[w])))
                                   + list(range(int(bwbase[w]), int(bwbase[w] + T_wB[w]))))
                            ph1 = p1pool.tile([D, WIN], mybir.dt.float32, space="PSUM")
                            for k, gt in enumerate(gts):
                                oh = ohpool.tile([P, WIN], BF16)
                                nc.vector.tensor_scalar(
                                    out=oh[:], in0=iota_b[:],
                                    scalar1=dst_t[:, gt:gt + 1],
                                    scalar2=w_t[:, gt:gt + 1],
                                    op0=mybir.AluOpType.is_equal,
                                    op1=mybir.AluOpType.mult,
                                )
                                j = gt - gtb
                                nc.tensor.matmul(
                                    out=ph1[:], lhsT=m[:, j * D:(j + 1) * D],
                                    rhs=oh[:], start=(k == 0),
                                    stop=(k == len(gts) - 1))
                            nc.vector.tensor_copy(
                                hT[:, (w - ph0) * WIN:(w - ph0 + 1) * WIN], ph1[:])
                        gw = ph1w - ph0
                        ph2 = p2pool.tile([D, PH * WIN], mybir.dt.float32, space="PSUM")
                        nc.tensor.matmul(out=ph2[:, :gw * WIN], lhsT=Wt[:],
                                         rhs=hT[:, :gw * WIN], start=True, stop=True)
                        nc.scalar.activation(
                            ow[:, (ph0 - g0) * WIN:(ph0 - g0 + gw) * WIN],
                            ph2[:, :gw * WIN],
                            mybir.ActivationFunctionType.Relu,
                            bias=b_t[:, 0:1], scale=1.0)
                    nc.sync.dma_start(out[:, g0 * WIN:g1 * WIN],
                                      ow[:, :(g1 - g0) * WIN])
    nc.compile()
    return nc


_CACHE = {}


def kernel(feature, W, b, src, dst):
    feature = np.asarray(feature, dtype=np.float32)
    W = np.asarray(W, dtype=np.float32)
    b = np.asarray(b, dtype=np.float32)
    src = np.asarray(src, dtype=np.int64)
    dst = np.asarray(dst, dtype=np.int64)

    in_maps, T_w, NT = _host_schedule(feature, W, b, src, dst)
    key = (NT, tuple(T_w.tolist()))
    if key not in _CACHE:
        _CACHE[key] = _build(T_w, NT)
    nc = _CACHE[key]
    res = run_bass_kernel_spmd(nc, in_maps, core_ids=list(range(N_CORES)))
    out = np.empty((N_NODES, D), dtype=np.float32)
    for c in range(N_CORES):
        lo = c * NPC
        hi = min(lo + NPC, N_NODES)
        oT = np.asarray(res.results[c]["out"]).astype(np.float32)  # [D, NPC]
        out[lo:hi] = oT.T[: hi - lo]
    return out
